# revision 15
# baseline (speedup 1.0000x reference)
"""AttentionPooling (segment softmax-pool) Trainium2 Bass kernel.

Full-input contract: kernel(**inputs) takes the unsharded inputs and
returns the full [1024, 256] float32 output. Internally shards 1024
graphs across 8 NeuronCores (128 contiguous graphs each, node ranges
padded to a common length) and runs one SPMD Bass/Tile kernel.

Math per core (one pass over x):
  h   = tanh(x @ W1 + b1)            # PE (bf16) + ACT, [hidden, node] layout
  s   = h @ W2                        # PE, N=1 matmuls -> scores as columns
  e   = exp(s + b2)                   # ACT
  scat[i, seg] = (batchloc[i]==seg)*e # DVE tensor_scalar (is_equal, mult)
  acc[seg, 0:256] += scat.T @ x       # PE, PSUM accumulate across all tiles
  acc[seg, 256]   += scat.T @ 1       # fused via ones column of x_aug
  out[seg] = acc[seg, 0:256] / (acc[seg, 256] + 1e-8)

Skipping the segment-max subtraction is numerically safe here: |s| is
bounded by ||W2||_1 + |b2| (~12), so exp never overflows fp32.

Data is shipped pre-swizzled from the host so every steady-state DMA is
one contiguous [128, 4112]-bf16 block (~1 MiB) per 1024-node supergroup:
per partition p: [xa(group even) 4x258 | xa(group odd) 4x258 | xT-half0
row p 1024 | xT-half1 row p 1024]. The xa part keys partitions by node
(for the scatter matmul); the xT part keys partitions by feature (for
the MLP matmuls).
"""

import os
from contextlib import ExitStack

import ml_dtypes
import numpy as np

N_CORES = 8
NUM_GRAPHS = 1024
BL = NUM_GRAPHS // N_CORES  # local segments per core = 128
HIDDEN = 256
HH = 128  # mlp hidden
P = 128
GROUP = 512  # nodes per compute group (4 tiles of 128)
SUPER = 1024  # nodes per DMA supergroup (2 compute groups)
XW = 258  # x row width in xa block: 256 features + ones col + pad col
SBL = 64  # segments per virtual shard (2 virtual shards per core)
XA_BLK = 4 * XW  # 1032 elems per compute group per partition
XC_W = 2 * XA_BLK + 2 * SUPER  # 4112 elems per partition per supergroup


def _build_bass(npad: int):
    import concourse.bacc as bacc
    import concourse.mybir as mybir
    import concourse.tile as tile

    dt = mybir.dt
    G = npad // GROUP
    Gd = npad // SUPER
    T = npad // P

    nc = bacc.Bacc("TRN2", target_bir_lowering=False, debug=False)

    xct = nc.dram_tensor("xct", [Gd, P, 2 * SUPER], dt.bfloat16, kind="ExternalInput")
    xca = nc.dram_tensor("xca", [Gd, P, 2 * XA_BLK], dt.bfloat16, kind="ExternalInput")
    blh = nc.dram_tensor("blh", [P, T], dt.float32, kind="ExternalInput")
    w1 = nc.dram_tensor("w1", [2, P, HH], dt.bfloat16, kind="ExternalInput")
    w2 = nc.dram_tensor("w2", [HH, 1], dt.bfloat16, kind="ExternalInput")
    b1 = nc.dram_tensor("b1", [HH, 1], dt.float32, kind="ExternalInput")
    b2c = nc.dram_tensor("b2c", [P, 1], dt.float32, kind="ExternalInput")
    iota = nc.dram_tensor("iota", [P, SBL], dt.bfloat16, kind="ExternalInput")
    out = nc.dram_tensor("out", [BL, HIDDEN], dt.float32, kind="ExternalOutput")

    with tile.TileContext(nc) as tc, ExitStack() as ctx:
        const = ctx.enter_context(tc.tile_pool(name="const", bufs=1))
        xt_pool = ctx.enter_context(tc.tile_pool(name="xt", bufs=6))
        xa_pool = ctx.enter_context(tc.tile_pool(name="xa", bufs=7))
        th_pool = ctx.enter_context(tc.tile_pool(name="th", bufs=4))
        e_pool = ctx.enter_context(tc.tile_pool(name="e", bufs=4))
        scat_pool = ctx.enter_context(tc.tile_pool(name="scat", bufs=32))
        fin_pool = ctx.enter_context(tc.tile_pool(name="fin", bufs=1))
        ph_pool = ctx.enter_context(tc.tile_pool(name="ph", bufs=2, space="PSUM"))
        ps_pool = ctx.enter_context(tc.tile_pool(name="ps", bufs=2, space="PSUM"))
        acc_pool = ctx.enter_context(tc.tile_pool(name="acc", bufs=1, space="PSUM"))

        warm_pool_t = const.tile([P, 16], dt.bfloat16, tag="warmdma0")
        nc.sync.dma_start(warm_pool_t[:], xct[0, :, 0:16])
        warm_pool_a = const.tile([P, 16], dt.bfloat16, tag="warmdma1")
        nc.scalar.dma_start(warm_pool_a[:], xca[0, :, 0:16])
        first_xt = xt_pool.tile([P, 2 * SUPER], dt.bfloat16)
        nc.sync.dma_start(first_xt[:], xct[0])

        # constants (all contiguous host layouts)
        w1_sb = const.tile([P, 2, HH], dt.bfloat16)
        nc.sync.dma_start(w1_sb[:], w1[:].rearrange("h k m -> k h m"))
        w2_sb = const.tile([HH, 1], dt.bfloat16)
        nc.sync.dma_start(w2_sb[:], w2[:])
        b1_sb = const.tile([HH, 1], dt.float32)
        nc.sync.dma_start(b1_sb[:], b1[:])
        b2_sb = const.tile([P, 1], dt.float32)
        nc.sync.dma_start(b2_sb[:], b2c[:])
        iota_sb = const.tile([P, SBL], dt.bfloat16)
        nc.sync.dma_start(iota_sb[:], iota[:])
        bl_sb = const.tile([P, T], dt.float32)
        nc.sync.dma_start(bl_sb[:], blh[:])

        acc_a = acc_pool.tile([SBL, XW], dt.float32)
        acc_b = acc_pool.tile([SBL, XW], dt.float32)
        t_half = (npad // 2) // P

        def finalize(k):
            # out = acc[:, 0:256] / (acc[:, 256] + 1e-8) for virtual shard k
            acc = (acc_a, acc_b)[k]
            sw1 = fin_pool.tile([SBL, 1], dt.float32, tag=f"sw{k}")
            nc.vector.tensor_scalar_add(sw1[:], acc[:, HIDDEN : HIDDEN + 1], 1e-8)
            recip = fin_pool.tile([SBL, 1], dt.float32, tag=f"rc{k}")
            nc.vector.reciprocal(recip[:], sw1[:])
            outf = fin_pool.tile([SBL, HIDDEN], dt.float32, tag=f"of{k}")
            nc.vector.tensor_scalar_mul(outf[:], acc[:, 0:HIDDEN], recip[:, 0:1])
            nc.sync.dma_start(out[k * SBL : (k + 1) * SBL, :], outf[:])

        # PE warm-up: dummy matmuls on the (tiny, early-arriving) weight
        # consts fill the initial xc-DMA wait and bring HAM to K=8/8 before
        # the first real matmul.
        for _ in range(24):
            wp = ph_pool.tile([HH, 2 * GROUP], dt.float32, tag="psum_h")
            nc.tensor.matmul(
                wp[:, 0 : 2 * HH], lhsT=w1_sb[:, 0, :], rhs=w1_sb[:, 0:2, :],
                start=True, stop=True,
            )

        n_tiles = G * 4
        xcts = {}

        xtts = {}
        xats = {}

        def dma_load_xt(d):
            t = xt_pool.tile([P, 2 * SUPER], dt.bfloat16)
            nc.sync.dma_start(t[:], xct[d])
            xtts[d] = t

        def dma_load_xa(d):
            t = xa_pool.tile([P, 2 * XA_BLK], dt.bfloat16)
            nc.scalar.dma_start(t[:], xca[d])
            xats[d] = t

        def xa_slice(g, s):
            t = xats[g // 2]
            base = (g % 2) * XA_BLK + s * XW
            return t[:, base : base + XW]

        def xt_slice(g, h):
            t = xtts[g // 2]
            base = h * SUPER + (g % 2) * GROUP
            return t[:, base : base + GROUP]

        ths = {}
        e4s = {}
        scats = {}

        PREFETCH = 4
        xtts[0] = first_xt
        dma_load_xa(0)
        for d in range(1, min(PREFETCH, Gd)):
            dma_load_xt(d)
            dma_load_xa(d)

        for dd in range(Gd + 3):
            d_next = dd + PREFETCH
            if d_next < Gd:
                dma_load_xt(d_next)
                dma_load_xa(d_next)

            if dd < Gd:
                psum_h = ph_pool.tile([HH, 2 * GROUP], dt.float32)
                for q in (0, 1):
                    g = 2 * dd + q
                    sl = slice(q * GROUP, (q + 1) * GROUP)
                    nc.tensor.matmul(
                        psum_h[:, sl], lhsT=w1_sb[:, 0, :], rhs=xt_slice(g, 0),
                        start=True, stop=False,
                    )
                    nc.tensor.matmul(
                        psum_h[:, sl], lhsT=w1_sb[:, 1, :], rhs=xt_slice(g, 1),
                        start=False, stop=True,
                    )
                th = th_pool.tile([HH, 2 * GROUP], dt.bfloat16)
                nc.scalar.activation(
                    th[:], psum_h[:], mybir.ActivationFunctionType.Tanh,
                    bias=b1_sb[:, 0:1], scale=1.0,
                )
                ths[dd] = th

            if 1 <= dd <= Gd:
                d1 = dd - 1
                xtts.pop(d1, None)
                th = ths.pop(d1)
                psum_s = ps_pool.tile([P, 8], dt.float32)
                for si in range(8):
                    nc.tensor.matmul(
                        psum_s[:, si : si + 1],
                        lhsT=th[:, si * P : (si + 1) * P],
                        rhs=w2_sb[:],
                        start=True, stop=True,
                    )
                e8 = e_pool.tile([P, 8], dt.float32)
                nc.scalar.activation(
                    e8[:], psum_s[:], mybir.ActivationFunctionType.Exp,
                    bias=b2_sb[:, 0:1], scale=1.0,
                )
                for q in (0, 1):
                    g = 2 * d1 + q
                    row = []
                    for sx in range(4):
                        t = g * 4 + sx
                        scat = scat_pool.tile([P, SBL], dt.bfloat16)
                        nc.vector.tensor_scalar(
                            out=scat[:],
                            in0=iota_sb[:],
                            scalar1=bl_sb[:, t : t + 1],
                            scalar2=e8[:, q * 4 + sx : q * 4 + sx + 1],
                            op0=mybir.AluOpType.is_equal,
                            op1=mybir.AluOpType.mult,
                        )
                        row.append(scat)
                    scats[g] = row

            if 3 <= dd:
                d2 = dd - 3
                for q in (0, 1):
                    g = 2 * d2 + q
                    row = scats.pop(g)
                    for s in range(4):
                        t = g * 4 + s
                        acc = acc_a if t < t_half else acc_b
                        nc.tensor.matmul(
                            acc[:],
                            lhsT=row[s][:],
                            rhs=xa_slice(g, s),
                            start=(t == 0 or t == t_half),
                            stop=(t == t_half - 1 or t == n_tiles - 1),
                            skip_group_check=True,
                        )
                xtts.pop(d2, None)
                xats.pop(d2)
                if d2 == t_half // 8 - 1:
                    finalize(0)

        finalize(1)

    nc.compile()
    return nc


def _maybe_enable_trace():
    """Dev-only NTFF profiling: register the axon NTFF hook if available.
    Inert when ATT_POOL_TRACE is unset (the grading path)."""
    if os.environ.get("ATT_POOL_TRACE") != "1":
        return False
    try:
        import sys
        import types

        import trn_agent_boot.trn_boot as tb

        hook = tb._ntff_profile_via_ctypes("/opt/axon/libaxon_pjrt.so")
        mod = types.ModuleType("antenv.axon_hooks")
        mod.get_axon_ntff_profile_hook = lambda: hook
        mod.set_axon_ntff_profile_hook = lambda h: None
        sys.modules["antenv.axon_hooks"] = mod

        import concourse.bass_utils as bu

        bu.upload_artifacts = lambda tmpdir: "local://" + str(tmpdir)
        return True
    except Exception as e:  # pragma: no cover - dev path only
        print("trace setup failed:", e)
        return False


def kernel(x, batch, W1, b1, W2, b2):
    from concourse.bass_utils import run_bass_kernel_spmd

    x = np.asarray(x, dtype=np.float32)
    batch = np.asarray(batch).astype(np.int64)
    W1 = np.asarray(W1, dtype=np.float32)
    b1 = np.asarray(b1, dtype=np.float32)
    W2 = np.asarray(W2, dtype=np.float32)
    b2 = np.asarray(b2, dtype=np.float32)

    bf16 = ml_dtypes.bfloat16

    bounds = np.searchsorted(batch, np.arange(0, NUM_GRAPHS + 1, SBL))
    shard = np.diff(bounds)
    npad_half = int(-(-int(shard.max()) // SUPER) * SUPER)
    npad = 2 * npad_half
    G = npad // GROUP
    Gd = npad // SUPER
    T = npad // P

    x_bf = x.astype(bf16)
    xct_h = np.zeros((N_CORES, Gd, P, 2 * SUPER), dtype=bf16)
    xca_h = np.zeros((N_CORES, Gd, P, 2 * XA_BLK), dtype=bf16)
    blh = np.full((N_CORES, P, T), 300.0, dtype=np.float32)
    for c in range(N_CORES):
        xa = np.zeros((npad, XW), dtype=bf16)
        xt = np.zeros((2, P, npad), dtype=bf16)
        bl = np.full(npad, 300.0, dtype=np.float32)
        for hh in range(2):
            idx = 2 * c + hh
            s0, s1 = int(bounds[idx]), int(bounds[idx + 1])
            n = s1 - s0
            o = hh * npad_half
            xa[o : o + n, :HIDDEN] = x_bf[s0:s1]
            xa[o : o + n, HIDDEN] = 1.0
            xt[0, :, o : o + n] = x_bf[s0:s1, 0:P].T
            xt[1, :, o : o + n] = x_bf[s0:s1, P:HIDDEN].T
            bl[o : o + n] = (batch[s0:s1] - idx * SBL).astype(np.float32)
        xa_sw = (
            xa.reshape(Gd, 2, 4, P, XW)
            .transpose(0, 3, 1, 2, 4)
            .reshape(Gd, P, 2 * XA_BLK)
        )
        xca_h[c] = xa_sw
        xt_sw = (
            xt.reshape(2, P, Gd, SUPER).transpose(2, 1, 0, 3).reshape(Gd, P, 2 * SUPER)
        )
        xct_h[c] = xt_sw
        blh[c] = bl.reshape(T, P).T
    w1_bf = np.ascontiguousarray(W1.astype(bf16).reshape(2, P, HH))
    w2_bf = np.ascontiguousarray(W2.astype(bf16).reshape(HH, 1))
    b1_col = np.ascontiguousarray(b1.reshape(HH, 1))
    b2_col = np.full((P, 1), float(b2[0]), dtype=np.float32)
    iota = np.ascontiguousarray(
        np.broadcast_to(np.arange(SBL, dtype=np.float32), (P, SBL))
    ).astype(bf16)

    trace = _maybe_enable_trace()
    nc = _build_bass(npad)

    in_maps = []
    for c in range(N_CORES):
        in_maps.append(
            {
                "xct": xct_h[c],
                "xca": xca_h[c],
                "blh": blh[c],
                "w1": w1_bf,
                "w2": w2_bf,
                "b1": b1_col,
                "b2c": b2_col,
                "iota": iota,
            }
        )

    res = run_bass_kernel_spmd(
        nc, in_maps, core_ids=list(range(N_CORES)), trace=trace
    )
    if trace and res.exec_time_ns is not None:
        print(f"HW exec time: {res.exec_time_ns} ns")
        if res.instructions_and_trace:
            print("trace:", res.instructions_and_trace[1])
        if res.profile_json:
            print("profile_json:", res.profile_json)

    out = np.concatenate([res.results[c]["out"] for c in range(N_CORES)], axis=0)
    assert out.shape == (NUM_GRAPHS, HIDDEN)
    return np.ascontiguousarray(out.astype(np.float32))


# revision 16
# speedup vs baseline: 1.0064x; 1.0064x over previous
"""AttentionPooling (segment softmax-pool) Trainium2 Bass kernel.

Full-input contract: kernel(**inputs) takes the unsharded inputs and
returns the full [1024, 256] float32 output. Internally shards 1024
graphs across 8 NeuronCores (128 contiguous graphs each, node ranges
padded to a common length) and runs one SPMD Bass/Tile kernel.

Math per core (one pass over x):
  h   = tanh(x @ W1 + b1)            # PE (bf16) + ACT, [hidden, node] layout
  s   = h @ W2                        # PE, N=1 matmuls -> scores as columns
  e   = exp(s + b2)                   # ACT
  scat[i, seg] = (batchloc[i]==seg)*e # DVE tensor_scalar (is_equal, mult)
  acc[seg, 0:256] += scat.T @ x       # PE, PSUM accumulate across all tiles
  acc[seg, 256]   += scat.T @ 1       # fused via ones column of x_aug
  out[seg] = acc[seg, 0:256] / (acc[seg, 256] + 1e-8)

Skipping the segment-max subtraction is numerically safe here: |s| is
bounded by ||W2||_1 + |b2| (~12), so exp never overflows fp32.

Data is shipped pre-swizzled from the host so every steady-state DMA is
one contiguous [128, 4112]-bf16 block (~1 MiB) per 1024-node supergroup:
per partition p: [xa(group even) 4x258 | xa(group odd) 4x258 | xT-half0
row p 1024 | xT-half1 row p 1024]. The xa part keys partitions by node
(for the scatter matmul); the xT part keys partitions by feature (for
the MLP matmuls).
"""

import os
from contextlib import ExitStack

import ml_dtypes
import numpy as np

N_CORES = 8
NUM_GRAPHS = 1024
BL = NUM_GRAPHS // N_CORES  # local segments per core = 128
HIDDEN = 256
HH = 128  # mlp hidden
P = 128
GROUP = 512  # nodes per compute group (4 tiles of 128)
SUPER = 1024  # nodes per DMA supergroup (2 compute groups)
XW = 258  # x row width in xa block: 256 features + ones col + pad col
SBL = 64  # segments per virtual shard (2 virtual shards per core)
XA_BLK = 4 * XW  # 1032 elems per compute group per partition
XC_W = 2 * XA_BLK + 2 * SUPER  # 4112 elems per partition per supergroup


def _build_bass(npad: int):
    import concourse.bacc as bacc
    import concourse.mybir as mybir
    import concourse.tile as tile

    dt = mybir.dt
    G = npad // GROUP
    Gd = npad // SUPER
    T = npad // P

    nc = bacc.Bacc("TRN2", target_bir_lowering=False, debug=False)

    xct = nc.dram_tensor("xct", [Gd, P, 2 * SUPER], dt.bfloat16, kind="ExternalInput")
    xca = nc.dram_tensor("xca", [Gd, P, 2 * XA_BLK], dt.bfloat16, kind="ExternalInput")
    blh = nc.dram_tensor("blh", [P, T], dt.float32, kind="ExternalInput")
    w1 = nc.dram_tensor("w1", [2, P, HH], dt.bfloat16, kind="ExternalInput")
    w2 = nc.dram_tensor("w2", [HH, 1], dt.bfloat16, kind="ExternalInput")
    b1 = nc.dram_tensor("b1", [HH, 1], dt.float32, kind="ExternalInput")
    b2c = nc.dram_tensor("b2c", [P, 1], dt.float32, kind="ExternalInput")
    iota = nc.dram_tensor("iota", [P, SBL], dt.bfloat16, kind="ExternalInput")
    out = nc.dram_tensor("out", [BL, HIDDEN], dt.float32, kind="ExternalOutput")

    with tile.TileContext(nc) as tc, ExitStack() as ctx:
        const = ctx.enter_context(tc.tile_pool(name="const", bufs=1))
        xt_pool = ctx.enter_context(tc.tile_pool(name="xt", bufs=6))
        xa_pool = ctx.enter_context(tc.tile_pool(name="xa", bufs=7))
        th_pool = ctx.enter_context(tc.tile_pool(name="th", bufs=4))
        e_pool = ctx.enter_context(tc.tile_pool(name="e", bufs=4))
        scat_pool = ctx.enter_context(tc.tile_pool(name="scat", bufs=32))
        fin_pool = ctx.enter_context(tc.tile_pool(name="fin", bufs=1))
        ph_pool = ctx.enter_context(tc.tile_pool(name="ph", bufs=2, space="PSUM"))
        ps_pool = ctx.enter_context(tc.tile_pool(name="ps", bufs=2, space="PSUM"))
        acc_pool = ctx.enter_context(tc.tile_pool(name="acc", bufs=1, space="PSUM"))

        first_xt = xt_pool.tile([P, 2 * SUPER], dt.bfloat16)
        nc.sync.dma_start(first_xt[:], xct[0])

        # constants (all contiguous host layouts)
        w1_sb = const.tile([P, 2, HH], dt.bfloat16)
        nc.sync.dma_start(w1_sb[:], w1[:].rearrange("h k m -> k h m"))
        w2_sb = const.tile([HH, 1], dt.bfloat16)
        nc.sync.dma_start(w2_sb[:], w2[:])
        b1_sb = const.tile([HH, 1], dt.float32)
        nc.sync.dma_start(b1_sb[:], b1[:])
        b2_sb = const.tile([P, 1], dt.float32)
        nc.sync.dma_start(b2_sb[:], b2c[:])
        iota_sb = const.tile([P, SBL], dt.bfloat16)
        nc.sync.dma_start(iota_sb[:], iota[:])
        bl_sb = const.tile([P, T], dt.float32)
        nc.sync.dma_start(bl_sb[:], blh[:])

        acc_a = acc_pool.tile([SBL, XW], dt.float32)
        acc_b = acc_pool.tile([SBL, XW], dt.float32)
        t_half = (npad // 2) // P

        def finalize(k):
            # out = acc[:, 0:256] / (acc[:, 256] + 1e-8) for virtual shard k
            acc = (acc_a, acc_b)[k]
            sw1 = fin_pool.tile([SBL, 1], dt.float32, tag=f"sw{k}")
            nc.vector.tensor_scalar_add(sw1[:], acc[:, HIDDEN : HIDDEN + 1], 1e-8)
            recip = fin_pool.tile([SBL, 1], dt.float32, tag=f"rc{k}")
            nc.vector.reciprocal(recip[:], sw1[:])
            outf = fin_pool.tile([SBL, HIDDEN], dt.float32, tag=f"of{k}")
            nc.vector.tensor_scalar_mul(outf[:], acc[:, 0:HIDDEN], recip[:, 0:1])
            nc.sync.dma_start(out[k * SBL : (k + 1) * SBL, :], outf[:])

        # PE warm-up: dummy matmuls on the (tiny, early-arriving) weight
        # consts fill the initial xc-DMA wait and bring HAM to K=8/8 before
        # the first real matmul.
        for _ in range(24):
            wp = ph_pool.tile([HH, 2 * GROUP], dt.float32, tag="psum_h")
            nc.tensor.matmul(
                wp[:, 0 : 2 * HH], lhsT=w1_sb[:, 0, :], rhs=w1_sb[:, 0:2, :],
                start=True, stop=True,
            )

        n_tiles = G * 4
        xcts = {}

        xtts = {}
        xats = {}

        def dma_load_xt(d):
            t = xt_pool.tile([P, 2 * SUPER], dt.bfloat16)
            nc.sync.dma_start(t[:], xct[d])
            xtts[d] = t

        def dma_load_xa(d):
            t = xa_pool.tile([P, 2 * XA_BLK], dt.bfloat16)
            nc.scalar.dma_start(t[:], xca[d])
            xats[d] = t

        def xa_slice(g, s):
            t = xats[g // 2]
            base = (g % 2) * XA_BLK + s * XW
            return t[:, base : base + XW]

        def xt_slice(g, h):
            t = xtts[g // 2]
            base = h * SUPER + (g % 2) * GROUP
            return t[:, base : base + GROUP]

        ths = {}
        e4s = {}
        scats = {}

        PREFETCH = 4
        xtts[0] = first_xt
        dma_load_xa(0)
        for d in range(1, min(PREFETCH, Gd)):
            dma_load_xt(d)
            dma_load_xa(d)

        for dd in range(Gd + 3):
            d_next = dd + PREFETCH
            if d_next < Gd:
                dma_load_xt(d_next)
                dma_load_xa(d_next)

            if dd < Gd:
                psum_h = ph_pool.tile([HH, 2 * GROUP], dt.float32)
                for q in (0, 1):
                    g = 2 * dd + q
                    sl = slice(q * GROUP, (q + 1) * GROUP)
                    nc.tensor.matmul(
                        psum_h[:, sl], lhsT=w1_sb[:, 0, :], rhs=xt_slice(g, 0),
                        start=True, stop=False,
                    )
                    nc.tensor.matmul(
                        psum_h[:, sl], lhsT=w1_sb[:, 1, :], rhs=xt_slice(g, 1),
                        start=False, stop=True,
                    )
                th = th_pool.tile([HH, 2 * GROUP], dt.bfloat16)
                nc.scalar.activation(
                    th[:], psum_h[:], mybir.ActivationFunctionType.Tanh,
                    bias=b1_sb[:, 0:1], scale=1.0,
                )
                ths[dd] = th

            if 1 <= dd <= Gd:
                d1 = dd - 1
                xtts.pop(d1, None)
                th = ths.pop(d1)
                psum_s = ps_pool.tile([P, 8], dt.float32)
                for si in range(8):
                    nc.tensor.matmul(
                        psum_s[:, si : si + 1],
                        lhsT=th[:, si * P : (si + 1) * P],
                        rhs=w2_sb[:],
                        start=True, stop=True,
                    )
                e8 = e_pool.tile([P, 8], dt.float32)
                nc.scalar.activation(
                    e8[:], psum_s[:], mybir.ActivationFunctionType.Exp,
                    bias=b2_sb[:, 0:1], scale=1.0,
                )
                for q in (0, 1):
                    g = 2 * d1 + q
                    row = []
                    for sx in range(4):
                        t = g * 4 + sx
                        scat = scat_pool.tile([P, SBL], dt.bfloat16)
                        nc.vector.tensor_scalar(
                            out=scat[:],
                            in0=iota_sb[:],
                            scalar1=bl_sb[:, t : t + 1],
                            scalar2=e8[:, q * 4 + sx : q * 4 + sx + 1],
                            op0=mybir.AluOpType.is_equal,
                            op1=mybir.AluOpType.mult,
                        )
                        row.append(scat)
                    scats[g] = row

            if 3 <= dd:
                d2 = dd - 3
                for q in (0, 1):
                    g = 2 * d2 + q
                    row = scats.pop(g)
                    for s in range(4):
                        t = g * 4 + s
                        acc = acc_a if t < t_half else acc_b
                        nc.tensor.matmul(
                            acc[:],
                            lhsT=row[s][:],
                            rhs=xa_slice(g, s),
                            start=(t == 0 or t == t_half),
                            stop=(t == t_half - 1 or t == n_tiles - 1),
                            skip_group_check=True,
                        )
                xtts.pop(d2, None)
                xats.pop(d2)
                if d2 == t_half // 8 - 1:
                    finalize(0)

        finalize(1)

    nc.compile()
    return nc


def _maybe_enable_trace():
    """Dev-only NTFF profiling: register the axon NTFF hook if available.
    Inert when ATT_POOL_TRACE is unset (the grading path)."""
    if os.environ.get("ATT_POOL_TRACE") != "1":
        return False
    try:
        import sys
        import types

        import trn_agent_boot.trn_boot as tb

        hook = tb._ntff_profile_via_ctypes("/opt/axon/libaxon_pjrt.so")
        mod = types.ModuleType("antenv.axon_hooks")
        mod.get_axon_ntff_profile_hook = lambda: hook
        mod.set_axon_ntff_profile_hook = lambda h: None
        sys.modules["antenv.axon_hooks"] = mod

        import concourse.bass_utils as bu

        bu.upload_artifacts = lambda tmpdir: "local://" + str(tmpdir)
        return True
    except Exception as e:  # pragma: no cover - dev path only
        print("trace setup failed:", e)
        return False


def kernel(x, batch, W1, b1, W2, b2):
    from concourse.bass_utils import run_bass_kernel_spmd

    x = np.asarray(x, dtype=np.float32)
    batch = np.asarray(batch).astype(np.int64)
    W1 = np.asarray(W1, dtype=np.float32)
    b1 = np.asarray(b1, dtype=np.float32)
    W2 = np.asarray(W2, dtype=np.float32)
    b2 = np.asarray(b2, dtype=np.float32)

    bf16 = ml_dtypes.bfloat16

    bounds = np.searchsorted(batch, np.arange(0, NUM_GRAPHS + 1, SBL))
    shard = np.diff(bounds)
    npad_half = int(-(-int(shard.max()) // SUPER) * SUPER)
    npad = 2 * npad_half
    G = npad // GROUP
    Gd = npad // SUPER
    T = npad // P

    x_bf = x.astype(bf16)
    xct_h = np.zeros((N_CORES, Gd, P, 2 * SUPER), dtype=bf16)
    xca_h = np.zeros((N_CORES, Gd, P, 2 * XA_BLK), dtype=bf16)
    blh = np.full((N_CORES, P, T), 300.0, dtype=np.float32)
    for c in range(N_CORES):
        xa = np.zeros((npad, XW), dtype=bf16)
        xt = np.zeros((2, P, npad), dtype=bf16)
        bl = np.full(npad, 300.0, dtype=np.float32)
        for hh in range(2):
            idx = 2 * c + hh
            s0, s1 = int(bounds[idx]), int(bounds[idx + 1])
            n = s1 - s0
            o = hh * npad_half
            xa[o : o + n, :HIDDEN] = x_bf[s0:s1]
            xa[o : o + n, HIDDEN] = 1.0
            xt[0, :, o : o + n] = x_bf[s0:s1, 0:P].T
            xt[1, :, o : o + n] = x_bf[s0:s1, P:HIDDEN].T
            bl[o : o + n] = (batch[s0:s1] - idx * SBL).astype(np.float32)
        xa_sw = (
            xa.reshape(Gd, 2, 4, P, XW)
            .transpose(0, 3, 1, 2, 4)
            .reshape(Gd, P, 2 * XA_BLK)
        )
        xca_h[c] = xa_sw
        xt_sw = (
            xt.reshape(2, P, Gd, SUPER).transpose(2, 1, 0, 3).reshape(Gd, P, 2 * SUPER)
        )
        xct_h[c] = xt_sw
        blh[c] = bl.reshape(T, P).T
    w1_bf = np.ascontiguousarray(W1.astype(bf16).reshape(2, P, HH))
    w2_bf = np.ascontiguousarray(W2.astype(bf16).reshape(HH, 1))
    b1_col = np.ascontiguousarray(b1.reshape(HH, 1))
    b2_col = np.full((P, 1), float(b2[0]), dtype=np.float32)
    iota = np.ascontiguousarray(
        np.broadcast_to(np.arange(SBL, dtype=np.float32), (P, SBL))
    ).astype(bf16)

    trace = _maybe_enable_trace()
    nc = _build_bass(npad)

    in_maps = []
    for c in range(N_CORES):
        in_maps.append(
            {
                "xct": xct_h[c],
                "xca": xca_h[c],
                "blh": blh[c],
                "w1": w1_bf,
                "w2": w2_bf,
                "b1": b1_col,
                "b2c": b2_col,
                "iota": iota,
            }
        )

    res = run_bass_kernel_spmd(
        nc, in_maps, core_ids=list(range(N_CORES)), trace=trace
    )
    if trace and res.exec_time_ns is not None:
        print(f"HW exec time: {res.exec_time_ns} ns")
        if res.instructions_and_trace:
            print("trace:", res.instructions_and_trace[1])
        if res.profile_json:
            print("profile_json:", res.profile_json)

    out = np.concatenate([res.results[c]["out"] for c in range(N_CORES)], axis=0)
    assert out.shape == (NUM_GRAPHS, HIDDEN)
    return np.ascontiguousarray(out.astype(np.float32))


# revision 17
# speedup vs baseline: 1.0840x; 1.0771x over previous
"""AttentionPooling (segment softmax-pool) Trainium2 Bass kernel.

Full-input contract: kernel(**inputs) takes the unsharded inputs and
returns the full [1024, 256] float32 output. Internally shards 1024
graphs across 8 NeuronCores (128 contiguous graphs each, node ranges
padded to a common length) and runs one SPMD Bass/Tile kernel.

Math per core (one pass over x):
  h   = tanh(x @ W1 + b1)            # PE (bf16) + ACT, [hidden, node] layout
  s   = h @ W2                        # PE, N=1 matmuls -> scores as columns
  e   = exp(s + b2)                   # ACT
  scat[i, seg] = (batchloc[i]==seg)*e # DVE tensor_scalar (is_equal, mult)
  acc[seg, 0:256] += scat.T @ x       # PE, PSUM accumulate across all tiles
  acc[seg, 256]   += scat.T @ 1       # fused via ones column of x_aug
  out[seg] = acc[seg, 0:256] / (acc[seg, 256] + 1e-8)

Skipping the segment-max subtraction is numerically safe here: |s| is
bounded by ||W2||_1 + |b2| (~12), so exp never overflows fp32.

Data is shipped pre-swizzled from the host so every steady-state DMA is
one contiguous [128, 4112]-bf16 block (~1 MiB) per 1024-node supergroup:
per partition p: [xa(group even) 4x258 | xa(group odd) 4x258 | xT-half0
row p 1024 | xT-half1 row p 1024]. The xa part keys partitions by node
(for the scatter matmul); the xT part keys partitions by feature (for
the MLP matmuls).
"""

import os
from contextlib import ExitStack

import ml_dtypes
import numpy as np

N_CORES = 8
NUM_GRAPHS = 1024
BL = NUM_GRAPHS // N_CORES  # local segments per core = 128
HIDDEN = 256
HH = 128  # mlp hidden
P = 128
GROUP = 512  # nodes per compute group (4 tiles of 128)
SUPER = 1024  # nodes per DMA supergroup (2 compute groups)
XW = 258  # x row width in xa block: 256 features + ones col + pad col
SBL = 64  # segments per virtual shard (2 virtual shards per core)
XA_BLK = 4 * XW  # 1032 elems per compute group per partition
XC_W = 2 * XA_BLK + 2 * SUPER  # 4112 elems per partition per supergroup


def _build_bass(npad: int):
    import concourse.bacc as bacc
    import concourse.mybir as mybir
    import concourse.tile as tile

    dt = mybir.dt
    G = npad // GROUP
    Gd = npad // SUPER
    T = npad // P

    nc = bacc.Bacc("TRN2", target_bir_lowering=False, debug=False)

    xct = nc.dram_tensor("xct", [Gd, P, 2 * SUPER], dt.bfloat16, kind="ExternalInput")
    xca = nc.dram_tensor("xca", [Gd, P, 2 * XA_BLK], dt.bfloat16, kind="ExternalInput")
    blh = nc.dram_tensor("blh", [P, T], dt.float32, kind="ExternalInput")
    w1 = nc.dram_tensor("w1", [2, P, HH], dt.bfloat16, kind="ExternalInput")
    w2 = nc.dram_tensor("w2", [HH, 1], dt.bfloat16, kind="ExternalInput")
    b1 = nc.dram_tensor("b1", [HH, 1], dt.float32, kind="ExternalInput")
    b2c = nc.dram_tensor("b2c", [P, 1], dt.float32, kind="ExternalInput")
    iota = nc.dram_tensor("iota", [P, SBL], dt.bfloat16, kind="ExternalInput")
    out = nc.dram_tensor("out", [BL, HIDDEN], dt.float32, kind="ExternalOutput")

    with tile.TileContext(nc) as tc, ExitStack() as ctx:
        const = ctx.enter_context(tc.tile_pool(name="const", bufs=1))
        xt_pool = ctx.enter_context(tc.tile_pool(name="xt", bufs=6))
        xa_pool = ctx.enter_context(tc.tile_pool(name="xa", bufs=7))
        th_pool = ctx.enter_context(tc.tile_pool(name="th", bufs=4))
        e_pool = ctx.enter_context(tc.tile_pool(name="e", bufs=4))
        scat_pool = ctx.enter_context(tc.tile_pool(name="scat", bufs=32))
        fin_pool = ctx.enter_context(tc.tile_pool(name="fin", bufs=1))
        ph_pool = ctx.enter_context(tc.tile_pool(name="ph", bufs=2, space="PSUM"))
        ps_pool = ctx.enter_context(tc.tile_pool(name="ps", bufs=2, space="PSUM"))
        acc_pool = ctx.enter_context(tc.tile_pool(name="acc", bufs=1, space="PSUM"))

        first_xt = xt_pool.tile([P, 2 * SUPER], dt.bfloat16)
        nc.sync.dma_start(first_xt[:], xct[0])

        # constants (all contiguous host layouts)
        w1_sb = const.tile([P, 2, HH], dt.bfloat16)
        nc.sync.dma_start(w1_sb[:], w1[:].rearrange("h k m -> k h m"))
        w2_sb = const.tile([HH, 1], dt.bfloat16)
        nc.sync.dma_start(w2_sb[:], w2[:])
        b1_sb = const.tile([HH, 1], dt.float32)
        nc.sync.dma_start(b1_sb[:], b1[:])
        b2_sb = const.tile([P, 1], dt.float32)
        nc.sync.dma_start(b2_sb[:], b2c[:])
        iota_sb = const.tile([P, SBL], dt.bfloat16)
        nc.sync.dma_start(iota_sb[:], iota[:])
        bl_sb = const.tile([P, T], dt.float32)
        nc.sync.dma_start(bl_sb[:], blh[:])

        acc_a = acc_pool.tile([SBL, XW], dt.float32)
        acc_b = acc_pool.tile([SBL, XW], dt.float32)
        t_half = (npad // 2) // P

        def finalize(k):
            # out = acc[:, 0:256] / (acc[:, 256] + 1e-8) for virtual shard k
            acc = (acc_a, acc_b)[k]
            sw1 = fin_pool.tile([SBL, 1], dt.float32, tag=f"sw{k}")
            nc.vector.tensor_scalar_add(sw1[:], acc[:, HIDDEN : HIDDEN + 1], 1e-8)
            recip = fin_pool.tile([SBL, 1], dt.float32, tag=f"rc{k}")
            nc.vector.reciprocal(recip[:], sw1[:])
            outf = fin_pool.tile([SBL, HIDDEN], dt.float32, tag=f"of{k}")
            nc.vector.tensor_scalar_mul(outf[:], acc[:, 0:HIDDEN], recip[:, 0:1])
            nc.sync.dma_start(out[k * SBL : (k + 1) * SBL, :], outf[:])

        # PE warm-up: dummy matmuls on the (tiny, early-arriving) weight
        # consts fill the initial xc-DMA wait and bring HAM to K=8/8 before
        # the first real matmul.
        for _ in range(24):
            wp = ph_pool.tile([HH, 2 * GROUP], dt.float32, tag="psum_h")
            nc.tensor.matmul(
                wp[:, 0 : 2 * HH], lhsT=w1_sb[:, 0, :], rhs=w1_sb[:, 0:2, :],
                start=True, stop=True,
            )

        n_tiles = G * 4
        xcts = {}

        xtts = {}
        xats = {}

        def dma_load_xt(d):
            t = xt_pool.tile([P, 2 * SUPER], dt.bfloat16)
            nc.sync.dma_start(t[:], xct[d])
            xtts[d] = t

        def dma_load_xa(d):
            t = xa_pool.tile([P, 2 * XA_BLK], dt.bfloat16)
            nc.scalar.dma_start(t[:], xca[d])
            xats[d] = t

        def xa_slice(g, s):
            t = xats[g // 2]
            base = (g % 2) * XA_BLK + s * XW
            return t[:, base : base + XW]

        def xt_slice(g, h):
            t = xtts[g // 2]
            base = h * SUPER + (g % 2) * GROUP
            return t[:, base : base + GROUP]

        ths = {}
        e4s = {}
        scats = {}

        PREFETCH = 4
        xtts[0] = first_xt
        dma_load_xa(0)
        for d in range(1, min(PREFETCH, Gd)):
            dma_load_xt(d)
            dma_load_xa(d)

        for dd in range(Gd + 3):
            d_next = dd + PREFETCH
            if d_next < Gd:
                dma_load_xt(d_next)
                dma_load_xa(d_next)

            if dd < Gd:
                psum_h = ph_pool.tile([HH, 2 * GROUP], dt.float32)
                for q in (0, 1):
                    g = 2 * dd + q
                    sl = slice(q * GROUP, (q + 1) * GROUP)
                    nc.tensor.matmul(
                        psum_h[:, sl], lhsT=w1_sb[:, 0, :], rhs=xt_slice(g, 0),
                        start=True, stop=False,
                    )
                    nc.tensor.matmul(
                        psum_h[:, sl], lhsT=w1_sb[:, 1, :], rhs=xt_slice(g, 1),
                        start=False, stop=True,
                    )
                th = th_pool.tile([HH, 2 * GROUP], dt.bfloat16)
                nc.scalar.activation(
                    th[:], psum_h[:], mybir.ActivationFunctionType.Tanh,
                    bias=b1_sb[:, 0:1], scale=1.0,
                )
                ths[dd] = th

            if 1 <= dd <= Gd:
                d1 = dd - 1
                xtts.pop(d1, None)
                th = ths.pop(d1)
                psum_s = ps_pool.tile([P, 8], dt.float32)
                for si in range(8):
                    nc.tensor.matmul(
                        psum_s[:, si : si + 1],
                        lhsT=th[:, si * P : (si + 1) * P],
                        rhs=w2_sb[:],
                        start=True, stop=True,
                    )
                e8 = e_pool.tile([P, 8], dt.float32)
                nc.scalar.activation(
                    e8[:], psum_s[:], mybir.ActivationFunctionType.Exp,
                    bias=b2_sb[:, 0:1], scale=1.0,
                )
                for q in (0, 1):
                    g = 2 * d1 + q
                    row = []
                    for sx in range(4):
                        t = g * 4 + sx
                        scat = scat_pool.tile([P, SBL], dt.bfloat16)
                        nc.vector.tensor_scalar(
                            out=scat[:],
                            in0=iota_sb[:],
                            scalar1=bl_sb[:, t : t + 1],
                            scalar2=e8[:, q * 4 + sx : q * 4 + sx + 1],
                            op0=mybir.AluOpType.is_equal,
                            op1=mybir.AluOpType.mult,
                        )
                        row.append(scat)
                    scats[g] = row

            if 3 <= dd:
                d2 = dd - 3
                for q in (0, 1):
                    g = 2 * d2 + q
                    row = scats.pop(g)
                    for s in range(4):
                        t = g * 4 + s
                        acc = acc_a if t < t_half else acc_b
                        nc.tensor.matmul(
                            acc[:],
                            lhsT=row[s][:],
                            rhs=xa_slice(g, s),
                            start=(t == 0 or t == t_half),
                            stop=(t == t_half - 1 or t == n_tiles - 1),
                            skip_group_check=True,
                        )
                xtts.pop(d2, None)
                xats.pop(d2)

        finalize(0)
        finalize(1)

    nc.compile()
    return nc


def _maybe_enable_trace():
    """Dev-only NTFF profiling: register the axon NTFF hook if available.
    Inert when ATT_POOL_TRACE is unset (the grading path)."""
    if os.environ.get("ATT_POOL_TRACE") != "1":
        return False
    try:
        import sys
        import types

        import trn_agent_boot.trn_boot as tb

        hook = tb._ntff_profile_via_ctypes("/opt/axon/libaxon_pjrt.so")
        mod = types.ModuleType("antenv.axon_hooks")
        mod.get_axon_ntff_profile_hook = lambda: hook
        mod.set_axon_ntff_profile_hook = lambda h: None
        sys.modules["antenv.axon_hooks"] = mod

        import concourse.bass_utils as bu

        bu.upload_artifacts = lambda tmpdir: "local://" + str(tmpdir)
        return True
    except Exception as e:  # pragma: no cover - dev path only
        print("trace setup failed:", e)
        return False


def kernel(x, batch, W1, b1, W2, b2):
    from concourse.bass_utils import run_bass_kernel_spmd

    x = np.asarray(x, dtype=np.float32)
    batch = np.asarray(batch).astype(np.int64)
    W1 = np.asarray(W1, dtype=np.float32)
    b1 = np.asarray(b1, dtype=np.float32)
    W2 = np.asarray(W2, dtype=np.float32)
    b2 = np.asarray(b2, dtype=np.float32)

    bf16 = ml_dtypes.bfloat16

    bounds = np.searchsorted(batch, np.arange(0, NUM_GRAPHS + 1, SBL))
    shard = np.diff(bounds)
    npad_half = int(-(-int(shard.max()) // SUPER) * SUPER)
    npad = 2 * npad_half
    G = npad // GROUP
    Gd = npad // SUPER
    T = npad // P

    x_bf = x.astype(bf16)
    xct_h = np.zeros((N_CORES, Gd, P, 2 * SUPER), dtype=bf16)
    xca_h = np.zeros((N_CORES, Gd, P, 2 * XA_BLK), dtype=bf16)
    blh = np.full((N_CORES, P, T), 300.0, dtype=np.float32)
    for c in range(N_CORES):
        xa = np.zeros((npad, XW), dtype=bf16)
        xt = np.zeros((2, P, npad), dtype=bf16)
        bl = np.full(npad, 300.0, dtype=np.float32)
        for hh in range(2):
            idx = 2 * c + hh
            s0, s1 = int(bounds[idx]), int(bounds[idx + 1])
            n = s1 - s0
            o = hh * npad_half
            xa[o : o + n, :HIDDEN] = x_bf[s0:s1]
            xa[o : o + n, HIDDEN] = 1.0
            xt[0, :, o : o + n] = x_bf[s0:s1, 0:P].T
            xt[1, :, o : o + n] = x_bf[s0:s1, P:HIDDEN].T
            bl[o : o + n] = (batch[s0:s1] - idx * SBL).astype(np.float32)
        xa_sw = (
            xa.reshape(Gd, 2, 4, P, XW)
            .transpose(0, 3, 1, 2, 4)
            .reshape(Gd, P, 2 * XA_BLK)
        )
        xca_h[c] = xa_sw
        xt_sw = (
            xt.reshape(2, P, Gd, SUPER).transpose(2, 1, 0, 3).reshape(Gd, P, 2 * SUPER)
        )
        xct_h[c] = xt_sw
        blh[c] = bl.reshape(T, P).T
    w1_bf = np.ascontiguousarray(W1.astype(bf16).reshape(2, P, HH))
    w2_bf = np.ascontiguousarray(W2.astype(bf16).reshape(HH, 1))
    b1_col = np.ascontiguousarray(b1.reshape(HH, 1))
    b2_col = np.full((P, 1), float(b2[0]), dtype=np.float32)
    iota = np.ascontiguousarray(
        np.broadcast_to(np.arange(SBL, dtype=np.float32), (P, SBL))
    ).astype(bf16)

    trace = _maybe_enable_trace()
    nc = _build_bass(npad)

    in_maps = []
    for c in range(N_CORES):
        in_maps.append(
            {
                "xct": xct_h[c],
                "xca": xca_h[c],
                "blh": blh[c],
                "w1": w1_bf,
                "w2": w2_bf,
                "b1": b1_col,
                "b2c": b2_col,
                "iota": iota,
            }
        )

    res = run_bass_kernel_spmd(
        nc, in_maps, core_ids=list(range(N_CORES)), trace=trace
    )
    if trace and res.exec_time_ns is not None:
        print(f"HW exec time: {res.exec_time_ns} ns")
        if res.instructions_and_trace:
            print("trace:", res.instructions_and_trace[1])
        if res.profile_json:
            print("profile_json:", res.profile_json)

    out = np.concatenate([res.results[c]["out"] for c in range(N_CORES)], axis=0)
    assert out.shape == (NUM_GRAPHS, HIDDEN)
    return np.ascontiguousarray(out.astype(np.float32))
